# revision 33
# baseline (speedup 1.0000x reference)
"""Trainium2 Bass kernel for nn_AttentionLayer (B=64, S=512, F=256), 8 cores.

Reference computation (per batch b):
    scores = x1 @ Wq + x2 @ Wk          # [S, S]
    a = softmax(tanh(scores), axis=-1)   # softmax over u
    a2 = a @ Wv                          # [S, S]
    out = a2 * x1                        # elementwise
    out = out * rsqrt(max(sum_s out^2, eps))   # l2-normalize over axis s

Strategy: pure data parallelism — 8 batches per core, weights replicated.
Everything is computed in a TRANSPOSED layout ([t-or-u partitions, s free]):
the host feeds x1^T and x2^T so both matmul stages consume operands with the
contraction dim on partitions, the softmax denominator is computed with a
ones-vector matmul over partitions, and the l2-normalization reduction lands
on the free axis where the vector engine is cheap.  Stage-A matmuls run in
float32r (4x fp32 TensorE throughput); stage-C/rowsum run in bf16 (exp output
is bf16).  Output is produced transposed and untransposed on the host.

The batch loop is software-pipelined with a 1-batch skew: stage A (DMA +
scores matmuls + tanh/exp) for batch b is emitted before stage B/C (+epilogue)
of batch b-1, so the TensorEngine always has independent stage-A work while
the scalar/vector engines finish the previous batch's epilogue.
"""

import sys

sys.path.insert(0, "/opt/trn_rl_repo")

import numpy as np
import ml_dtypes

import concourse.bass as bass
import concourse.tile as tile
from concourse import bacc, mybir
from concourse.bass_utils import run_bass_kernel_spmd

B, S, F = 64, 512, 256
N_CORES = 8
BPC = B // N_CORES  # batches per core
P = 128
KT1 = S // P  # 4 k-tiles over t (x1/Wq contraction)
KT2 = F // P  # 2 k-tiles over f (x2/Wk contraction)
NT = S // P  # 4 m-tiles over u (stage A) / t (stage C)
EPS = 1e-12

F32 = mybir.dt.float32
F32R = mybir.dt.float32r
BF16 = mybir.dt.bfloat16
AF = mybir.ActivationFunctionType
ALU = mybir.AluOpType

last_results = None  # test harness introspection


def build_nc(reps=1, bpc=BPC):
    nc = bacc.Bacc(
        "TRN2", target_bir_lowering=False, debug=False, num_devices=N_CORES
    )
    # f32r params carry raw fp32 bits; the PE rounds internally.
    x1t = nc.declare_dram_parameter("x1t", [bpc, S, S], F32R, isOutput=False)
    x2t = nc.declare_dram_parameter("x2t", [bpc, F, S], F32R, isOutput=False)
    wq = nc.declare_dram_parameter("wq", [S, S], F32R, isOutput=False)
    wk = nc.declare_dram_parameter("wk", [F, S], F32R, isOutput=False)
    wv = nc.declare_dram_parameter("wv", [S, S], BF16, isOutput=False)
    out = nc.declare_dram_parameter("out", [bpc, S, S], F32, isOutput=True)

    x1t_r = x1t.ap().rearrange("b (a p) s -> b a p s", p=P)
    x2t_r = x2t.ap().rearrange("b (a p) s -> b a p s", p=P)
    out_r = out.ap().rearrange("b (a p) s -> b a p s", p=P)
    wq_r = wq.ap().rearrange("(a p) u -> a p u", p=P)
    wk_r = wk.ap().rearrange("(a p) u -> a p u", p=P)
    wv_r = wv.ap().rearrange("(a p) t -> a p t", p=P)

    batches = [bb for _ in range(reps) for bb in range(bpc)]

    with tile.TileContext(nc) as tc:
        with (
            tc.tile_pool(name="singles", bufs=1) as singles,
            tc.tile_pool(name="xin", bufs=1) as xin,
            tc.tile_pool(name="work", bufs=4) as work,
            tc.tile_pool(name="small", bufs=2) as small,
            tc.tile_pool(name="outp", bufs=3) as outp,
            tc.tile_pool(name="psA", bufs=2, space="PSUM") as psA,
            tc.tile_pool(name="psY", bufs=3, space="PSUM") as psY,
            tc.tile_pool(name="psR", bufs=1, space="PSUM") as psR,
        ):
            # First batch's x1 DMAs interleaved with weight DMAs so the first
            # matmul can start after ~2 small transfers.
            # Startup DMAs round-robin across engine queues so the first
            # matmuls aren't serialized behind 4MB on one queue.
            x1_first = []
            wq_sb, wk_sb, wv_sb = [], [], []
            b0 = batches[0]
            engs = [nc.scalar, nc.gpsimd, nc.scalar, nc.gpsimd]
            for kt in range(KT1):
                w_t = singles.tile([P, S], F32R, tag=f"wq{kt}")
                engs[kt % 4].dma_start(out=w_t, in_=wq_r[kt])
                wq_sb.append(w_t)
                x_t = xin.tile([P, S], F32R, tag="x1", bufs=4 * 4)
                nc.sync.dma_start(out=x_t, in_=x1t_r[b0, kt])
                x1_first.append(x_t)
            for kt in range(KT2):
                w_t = singles.tile([P, S], F32R, tag=f"wk{kt}")
                engs[kt % 4].dma_start(out=w_t, in_=wk_r[kt])
                wk_sb.append(w_t)
            for ut in range(NT):
                w_t = singles.tile([P, S], BF16, tag=f"wv{ut}")
                engs[ut % 4].dma_start(out=w_t, in_=wv_r[ut])
                wv_sb.append(w_t)
            ones_col = singles.tile([P, 1], BF16)
            nc.vector.memset(ones_col, 1.0)
            eps_t = singles.tile([P, 1], F32)
            nc.vector.memset(eps_t, EPS)

            def stage_a(b, x1_tiles, mid_cb=None):
                """DMA x2 (x1 tiles already DMA'd), scores matmuls (in u-tile
                pairs sharing one 2-bank PSUM tile), tanh+exp over pairs.
                mid_cb (if set) is emitted between the two pair-halves so the
                previous batch's softmax-denominator chain overlaps this
                batch's remaining matmuls."""
                x2_tiles = []
                x2_eng = nc.scalar if b % 2 == 0 else nc.sync
                for kt in range(KT2):
                    x_t = xin.tile([P, S], F32R, tag="x2", bufs=2 * 4)
                    x2_eng.dma_start(out=x_t, in_=x2t_r[b, kt])
                    x2_tiles.append(x_t)
                expz = work.tile([P, NT, S], BF16, tag="expz")
                for half in range(NT // 2):
                    sc = psA.tile([P, 2, S], F32, tag="scores")
                    for j in range(2):
                        ut = half * 2 + j
                        for kt in range(KT1):
                            nc.tensor.matmul(
                                sc[:, j, :],
                                wq_sb[kt][:, ut * P : (ut + 1) * P],
                                x1_tiles[kt],
                                start=(kt == 0),
                                stop=False,
                            )
                        for kt in range(KT2):
                            nc.tensor.matmul(
                                sc[:, j, :],
                                wk_sb[kt][:, ut * P : (ut + 1) * P],
                                x2_tiles[kt],
                                start=False,
                                stop=(kt == KT2 - 1),
                            )
                    tanh_t = work.tile([P, 2, S], F32, tag="tanh")
                    nc.scalar.activation(out=tanh_t, in_=sc, func=AF.Tanh)
                    nc.scalar.activation(
                        out=expz[:, half * 2 : half * 2 + 2, :],
                        in_=tanh_t,
                        func=AF.Exp,
                    )
                    if half == 0 and mid_cb is not None:
                        mid_cb()
                return expz

            def stage_b(b, expz):
                """softmax denominator: rowsum -> recip -> broadcast -> a."""
                rs = psR.tile([1, S], F32, tag="rowsum")
                for ut in range(NT):
                    nc.tensor.matmul(
                        rs,
                        ones_col,
                        expz[:, ut, :],
                        start=(ut == 0),
                        stop=(ut == NT - 1),
                    )
                recip = small.tile([1, S], BF16, tag="recip")
                with nc.allow_low_precision(reason="softmax recip to bf16 matches bf16 a"):
                    nc.vector.reciprocal(out=recip, in_=rs)
                bc = small.tile([P, S], BF16, tag="bc")
                nc.gpsimd.partition_broadcast(bc, recip)
                a_t = work.tile([P, NT, S], BF16, tag="a")
                for ut in range(NT):
                    nc.vector.tensor_tensor(
                        out=a_t[:, ut, :], in0=expz[:, ut, :], in1=bc, op=ALU.mult
                    )
                return a_t

            def stage_c(b, x1_tiles, a_t):
                """Y matmuls on normalized a, epilogue, out DMA."""
                out_sb = outp.tile([P, NT, S], F32, tag="out")
                sumsq = small.tile([P, NT], F32, tag="sumsq")
                for tt in range(NT):
                    y = psY.tile([P, S], F32, tag="y")
                    for ut in range(NT):
                        nc.tensor.matmul(
                            y,
                            wv_sb[ut][:, tt * P : (tt + 1) * P],
                            a_t[:, ut, :],
                            start=(ut == 0),
                            stop=(ut == NT - 1),
                        )
                    q_t = out_sb[:, tt, :]
                    nc.vector.tensor_tensor(
                        out=q_t, in0=y, in1=x1_tiles[tt].bitcast(F32), op=ALU.mult
                    )
                    scr = work.tile([P, S], F32, tag="scr")
                    nc.vector.scalar_tensor_tensor(
                        out=scr,
                        in0=q_t,
                        scalar=1.0,
                        in1=q_t,
                        op0=ALU.mult,
                        op1=ALU.mult,
                        accum_out=sumsq[:, tt : tt + 1],
                    )
                rsq = small.tile([P, NT], F32, tag="rsq")
                nc.scalar.activation(out=rsq, in_=sumsq, func=AF.Sqrt, bias=eps_t)
                for tt in range(NT):
                    nc.gpsimd.normalize_recip(
                        out_ap=out_sb[:, tt, :],
                        in_ap=out_sb[:, tt, :],
                        denom_ap=rsq[:, tt : tt + 1],
                    )
                    nc.sync.dma_start(out=out_r[b, tt], in_=out_sb[:, tt, :])


            pending = None  # (b, x1_tiles, expz) awaiting stages B+C
            for i, b in enumerate(batches):
                if i == 0:
                    x1_tiles = x1_first
                else:
                    x1_tiles = []
                    in_eng = nc.sync if i % 2 == 0 else nc.scalar
                    for kt in range(KT1):
                        x_t = xin.tile([P, S], F32R, tag="x1", bufs=4 * 4)
                        in_eng.dma_start(out=x_t, in_=x1t_r[b, kt])
                        x1_tiles.append(x_t)
                prev = pending
                hold = {}

                def mid_cb():
                    hold["a"] = stage_b(prev[0], prev[2])

                expz = stage_a(b, x1_tiles, mid_cb if prev is not None else None)
                if prev is not None:
                    stage_c(prev[0], prev[1], hold["a"])
                pending = (b, x1_tiles, expz)
            a_last = stage_b(pending[0], pending[2])
            stage_c(pending[0], pending[1], a_last)

    nc.compile()
    return nc


_nc_cache = None


def kernel(x1, x2, W_query, W_key, W_value, _trace=False):
    global _nc_cache, last_results
    x1 = np.ascontiguousarray(np.asarray(x1, dtype=np.float32).transpose(0, 2, 1))
    x2 = np.ascontiguousarray(np.asarray(x2, dtype=np.float32).transpose(0, 2, 1))
    wq = np.asarray(W_query, dtype=np.float32)
    wk = np.asarray(W_key, dtype=np.float32)
    wv = np.asarray(W_value, dtype=ml_dtypes.bfloat16)

    if _nc_cache is None:
        _nc_cache = build_nc()
    nc = _nc_cache

    in_maps = []
    for c in range(N_CORES):
        sl = slice(c * BPC, (c + 1) * BPC)
        in_maps.append(
            {"x1t": x1[sl], "x2t": x2[sl], "wq": wq, "wk": wk, "wv": wv}
        )
    res = run_bass_kernel_spmd(
        nc, in_maps, core_ids=list(range(N_CORES)), trace=_trace
    )
    last_results = res
    outT = np.concatenate([res.results[c]["out"] for c in range(N_CORES)], axis=0)
    return np.ascontiguousarray(outT.transpose(0, 2, 1))



# revision 34
# speedup vs baseline: 1.3366x; 1.3366x over previous
"""Trainium2 Bass kernel for nn_AttentionLayer (B=64, S=512, F=256), 8 cores.

Reference computation (per batch b):
    scores = x1 @ Wq + x2 @ Wk          # [S, S]
    a = softmax(tanh(scores), axis=-1)   # softmax over u
    a2 = a @ Wv                          # [S, S]
    out = a2 * x1                        # elementwise
    out = out * rsqrt(max(sum_s out^2, eps))   # l2-normalize over axis s

Strategy: pure data parallelism — 8 batches per core, weights replicated.
Everything is computed in a TRANSPOSED layout ([t-or-u partitions, s free]):
the host feeds x1^T and x2^T so both matmul stages consume operands with the
contraction dim on partitions, the softmax denominator is computed with a
ones-vector matmul over partitions, and the l2-normalization reduction lands
on the free axis where the vector engine is cheap.  Stage-A matmuls run in
float32r (4x fp32 TensorE throughput); stage-C/rowsum run in bf16 (exp output
is bf16).  Output is produced transposed and untransposed on the host.

The batch loop is software-pipelined with a 1-batch skew: stage A (DMA +
scores matmuls + tanh/exp) for batch b is emitted before stage B/C (+epilogue)
of batch b-1, so the TensorEngine always has independent stage-A work while
the scalar/vector engines finish the previous batch's epilogue.
"""

import sys

sys.path.insert(0, "/opt/trn_rl_repo")

import numpy as np
import ml_dtypes

import concourse.bass as bass
import concourse.tile as tile
from concourse import bacc, mybir
from concourse.bass_utils import run_bass_kernel_spmd

B, S, F = 64, 512, 256
N_CORES = 8
BPC = B // N_CORES  # batches per core
P = 128
KT1 = S // P  # 4 k-tiles over t (x1/Wq contraction)
KT2 = F // P  # 2 k-tiles over f (x2/Wk contraction)
NT = S // P  # 4 m-tiles over u (stage A) / t (stage C)
EPS = 1e-12

F32 = mybir.dt.float32
F32R = mybir.dt.float32r
BF16 = mybir.dt.bfloat16
AF = mybir.ActivationFunctionType
ALU = mybir.AluOpType

last_results = None  # test harness introspection


def build_nc(reps=1, bpc=BPC):
    nc = bacc.Bacc(
        "TRN2", target_bir_lowering=False, debug=False, num_devices=N_CORES
    )
    # f32r params carry raw fp32 bits; the PE rounds internally.
    x1t = nc.declare_dram_parameter("x1t", [bpc, S, S], F32R, isOutput=False)
    x2t = nc.declare_dram_parameter("x2t", [bpc, F, S], F32R, isOutput=False)
    wq = nc.declare_dram_parameter("wq", [S, S], F32R, isOutput=False)
    wk = nc.declare_dram_parameter("wk", [F, S], F32R, isOutput=False)
    wv = nc.declare_dram_parameter("wv", [S, S], BF16, isOutput=False)
    out = nc.declare_dram_parameter("out", [bpc, S, S], F32, isOutput=True)

    x1t_r = x1t.ap().rearrange("b (a p) s -> b a p s", p=P)
    x2t_r = x2t.ap().rearrange("b (a p) s -> b a p s", p=P)
    out_r = out.ap().rearrange("b (a p) s -> b a p s", p=P)
    wq_r = wq.ap().rearrange("(a p) u -> a p u", p=P)
    wk_r = wk.ap().rearrange("(a p) u -> a p u", p=P)
    wv_r = wv.ap().rearrange("(a p) t -> a p t", p=P)

    batches = [bb for _ in range(reps) for bb in range(bpc)]

    with tile.TileContext(nc) as tc:
        with (
            tc.tile_pool(name="singles", bufs=1) as singles,
            tc.tile_pool(name="xin", bufs=1) as xin,
            tc.tile_pool(name="work", bufs=4) as work,
            tc.tile_pool(name="small", bufs=2) as small,
            tc.tile_pool(name="outp", bufs=3) as outp,
            tc.tile_pool(name="psA", bufs=2, space="PSUM") as psA,
            tc.tile_pool(name="psY", bufs=3, space="PSUM") as psY,
            tc.tile_pool(name="psR", bufs=1, space="PSUM") as psR,
        ):
            # First batch's x1 DMAs interleaved with weight DMAs so the first
            # matmul can start after ~2 small transfers.
            # Startup DMAs round-robin across engine queues so the first
            # matmuls aren't serialized behind 4MB on one queue.
            wq_sb, wk_sb, wv_sb = [], [], []
            b0 = batches[0]
            engs = [nc.scalar, nc.gpsimd, nc.scalar, nc.gpsimd]
            x1_first = xin.tile([P, KT1, S], F32R, tag="x1", bufs=4)
            for kt in range(KT1):
                w_t = singles.tile([P, S], F32R, tag=f"wq{kt}")
                engs[kt % 4].dma_start(out=w_t, in_=wq_r[kt])
                wq_sb.append(w_t)
                nc.sync.dma_start(out=x1_first[:, kt, :], in_=x1t_r[b0, kt])
            for kt in range(KT2):
                w_t = singles.tile([P, S], F32R, tag=f"wk{kt}")
                engs[kt % 4].dma_start(out=w_t, in_=wk_r[kt])
                wk_sb.append(w_t)
            for ut in range(NT):
                w_t = singles.tile([P, S], BF16, tag=f"wv{ut}")
                engs[ut % 4].dma_start(out=w_t, in_=wv_r[ut])
                wv_sb.append(w_t)
            ones_col = singles.tile([P, 1], BF16)
            nc.vector.memset(ones_col, 1.0)
            eps_t = singles.tile([P, 1], F32)
            nc.vector.memset(eps_t, EPS)

            def stage_a(b, x1_sb, mid_cb=None):
                """DMA x2 (x1 tiles already DMA'd), scores matmuls (in u-tile
                pairs sharing one 2-bank PSUM tile), tanh+exp over pairs.
                mid_cb (if set) is emitted between the two pair-halves so the
                previous batch's softmax-denominator chain overlaps this
                batch's remaining matmuls."""
                x2_sb = xin.tile([P, KT2, S], F32R, tag="x2", bufs=4)
                for kt in range(KT2):
                    nc.sync.dma_start(out=x2_sb[:, kt, :], in_=x2t_r[b, kt])
                expz = work.tile([P, NT, S], BF16, tag="expz")
                for half in range(NT // 2):
                    sc = psA.tile([P, 2, S], F32, tag="scores")
                    for j in range(2):
                        ut = half * 2 + j
                        for kt in range(KT1):
                            nc.tensor.matmul(
                                sc[:, j, :],
                                wq_sb[kt][:, ut * P : (ut + 1) * P],
                                x1_sb[:, kt, :],
                                start=(kt == 0),
                                stop=False,
                            )
                        for kt in range(KT2):
                            nc.tensor.matmul(
                                sc[:, j, :],
                                wk_sb[kt][:, ut * P : (ut + 1) * P],
                                x2_sb[:, kt, :],
                                start=False,
                                stop=(kt == KT2 - 1),
                            )
                    tanh_t = work.tile([P, 2, S], F32, tag="tanh")
                    nc.scalar.activation(out=tanh_t, in_=sc, func=AF.Tanh)
                    nc.scalar.activation(
                        out=expz[:, half * 2 : half * 2 + 2, :],
                        in_=tanh_t,
                        func=AF.Exp,
                    )
                    if half == 0 and mid_cb is not None:
                        mid_cb()
                return expz

            def stage_b(b, expz):
                """softmax denominator: rowsum -> recip -> broadcast -> a."""
                rs = psR.tile([1, S], F32, tag="rowsum")
                for ut in range(NT):
                    nc.tensor.matmul(
                        rs,
                        ones_col,
                        expz[:, ut, :],
                        start=(ut == 0),
                        stop=(ut == NT - 1),
                    )
                recip_f = small.tile([1, S], F32, tag="recipf")
                nc.vector.reciprocal_approx_fast(out=recip_f, in_=rs)
                recip = small.tile([1, S], BF16, tag="recip")
                nc.vector.tensor_copy(out=recip, in_=recip_f)
                bc = small.tile([P, S], BF16, tag="bc")
                nc.gpsimd.partition_broadcast(bc, recip)
                a_t = work.tile([P, NT, S], BF16, tag="a")
                for ut in range(NT):
                    nc.vector.tensor_tensor(
                        out=a_t[:, ut, :], in0=expz[:, ut, :], in1=bc, op=ALU.mult
                    )
                return a_t

            def stage_c(b, x1_sb, a_t):
                """Y matmuls on normalized a, q = y*x1, sum-of-squares."""
                out_sb = outp.tile([P, NT, S], F32, tag="out")
                sumsq = small.tile([P, NT], F32, tag="sumsq", bufs=4)
                for tt in range(NT):
                    y = psY.tile([P, S], F32, tag="y")
                    for ut in range(NT):
                        nc.tensor.matmul(
                            y,
                            wv_sb[ut][:, tt * P : (tt + 1) * P],
                            a_t[:, ut, :],
                            start=(ut == 0),
                            stop=(ut == NT - 1),
                        )
                    q_t = out_sb[:, tt, :]
                    nc.vector.tensor_tensor(
                        out=q_t, in0=y, in1=x1_sb[:, tt, :].bitcast(F32), op=ALU.mult
                    )
                    scr = work.tile([P, S], F32, tag="scr")
                    nc.vector.scalar_tensor_tensor(
                        out=scr,
                        in0=q_t,
                        scalar=1.0,
                        in1=q_t,
                        op0=ALU.mult,
                        op1=ALU.mult,
                        accum_out=sumsq[:, tt : tt + 1],
                    )
                return out_sb, sumsq

            def stage_fin(b, out_sb, sumsq):
                """sqrt (ACT, emitted adjacently for pairs of batches to halve
                activation-table swaps), Pool divide, store."""
                rsq = small.tile([P, NT], F32, tag="rsq", bufs=4)
                nc.scalar.activation(out=rsq, in_=sumsq, func=AF.Sqrt, bias=eps_t)
                for tt in range(NT):
                    nc.gpsimd.normalize_recip(
                        out_ap=out_sb[:, tt, :],
                        in_ap=out_sb[:, tt, :],
                        denom_ap=rsq[:, tt : tt + 1],
                    )
                nc.scalar.dma_start(out=out_r[b].rearrange("a p s -> p a s"), in_=out_sb)


            pending = None  # (b, x1_sb, expz) awaiting stages B+C
            fins = []  # (b, out_sb, sumsq) awaiting finalize, flushed in pairs
            for i, b in enumerate(batches):
                if i == 0:
                    x1_sb = x1_first
                else:
                    x1_sb = xin.tile([P, KT1, S], F32R, tag="x1", bufs=4)
                    for kt in range(KT1):
                        nc.sync.dma_start(out=x1_sb[:, kt, :], in_=x1t_r[b, kt])
                prev = pending
                hold = {}

                def mid_cb():
                    hold["a"] = stage_b(prev[0], prev[2])

                expz = stage_a(b, x1_sb, mid_cb if prev is not None else None)
                if prev is not None:
                    fins.append((prev[0],) + stage_c(prev[0], prev[1], hold["a"]))
                    if len(fins) == 2:
                        for f in fins:
                            stage_fin(*f)
                        fins = []
                pending = (b, x1_sb, expz)
            a_last = stage_b(pending[0], pending[2])
            fins.append((pending[0],) + stage_c(pending[0], pending[1], a_last))
            for f in fins:
                stage_fin(*f)

    nc.compile()
    return nc


_nc_cache = None


def kernel(x1, x2, W_query, W_key, W_value, _trace=False):
    global _nc_cache, last_results
    x1 = np.ascontiguousarray(np.asarray(x1, dtype=np.float32).transpose(0, 2, 1))
    x2 = np.ascontiguousarray(np.asarray(x2, dtype=np.float32).transpose(0, 2, 1))
    wq = np.asarray(W_query, dtype=np.float32)
    wk = np.asarray(W_key, dtype=np.float32)
    wv = np.asarray(W_value, dtype=ml_dtypes.bfloat16)

    if _nc_cache is None:
        _nc_cache = build_nc()
    nc = _nc_cache

    in_maps = []
    for c in range(N_CORES):
        sl = slice(c * BPC, (c + 1) * BPC)
        in_maps.append(
            {"x1t": x1[sl], "x2t": x2[sl], "wq": wq, "wk": wk, "wv": wv}
        )
    res = run_bass_kernel_spmd(
        nc, in_maps, core_ids=list(range(N_CORES)), trace=_trace
    )
    last_results = res
    outT = np.concatenate([res.results[c]["out"] for c in range(N_CORES)], axis=0)
    return np.ascontiguousarray(outT.transpose(0, 2, 1))

